# revision 35
# baseline (speedup 1.0000x reference)
"""AffinityLoss (segment-reduce) Trainium2 kernel.

Math (single pass over the data — no per-row center gather needed):
    lbl     = argmax(labels, axis=1)                         (N,)
    sums_c  = sum of features rows with lbl == c             (C, D)
    n_c     = count of rows with lbl == c                    (C,)
    sumsq   = sum(features ** 2)                             scalar
    centers = where(n>0, sums/max(n,1), 0) + 1e-6
    intra   = sumsq - 2*sum(sums*centers) + sum(n_c*||c_c||^2)
    inter   = sum((centers - mean(centers))^2) / C
    loss    = intra / (inter + 1e-6)

Per core (data-parallel over N):
  - one-hot(argmax) built on the vector engine (reduce_max + one
    broadcast is_equal over the whole supertile)
  - segment sums via PE: one matmul per 128-row group
    (one-hot^T @ features) accumulated in PSUM over the full loop
  - counts via PE with ones as the stationary operand
    (ones^T @ one-hot -> per-(j,c) column counts)
  - sum-of-squares on the scalar engine (Square activation + accumulate)
Features stream as f32 -> bf16 cast DMAs (SWDGE), contiguous per
partition per supertile; the supertile schedule tapers at the end so the
compute tail after the last DMA is short. The O(C*D) finalization runs
on the host over the 8 per-core partials (the gather/unshard step).

Measured: ~131us HW exec on 8 cores (pure-DMA floor for the same
45.25MB/core stream: ~125us; HBM roofline 45.25MB / 358GB/s = 126us).
"""

import numpy as np

import concourse.bacc as bacc
import concourse.tile as tile
from concourse import mybir
from concourse.bass_utils import run_bass_kernel_spmd

N_CORES = 8
N_TOTAL = 262144
D = 256
C = 100
P = 128
T = 16  # 128-row groups per supertile (DMA batch)

F32 = mybir.dt.float32
BF16 = mybir.dt.bfloat16


def build_nc(
    rows_per_core: int,
    t: int = T,
    bufs: int = 6,
    tail_mode: int = 3,
    pool_cast: bool = False,
    lbl_on_act: bool = False,
):
    """Build the per-core Bass program (same SPMD program on all cores).

    tail_mode: 0 = uniform supertiles; 1 = tapered tail [t/2,t/4,t/4];
    3 = deep taper [t/2,t/4,t/8,t/8] (default; tiny tiles use one-shot
    counts matmuls); 2 = mode 3 plus up-front tail one-hots and split sums
    PSUM — measured consistently slower, kept for reference.
    """
    total_j = rows_per_core // P
    cc = 4  # j's per counts matmul (free dim cc*C <= 512)
    assert t % cc == 0
    # Supertile schedule: mostly t, tapering at the end so the compute tail
    # after the final DMA is short.
    if tail_mode >= 2 and t % 8 == 0:
        tail = [t // 2, t // 4, t // 8, t // 8]
    elif t // 4 >= cc:
        tail = [t // 2, t // 4, t // 4]
    else:
        tail = [t // 2, t // 2]
    if (
        tail_mode > 0
        and total_j > 2 * t
        and t >= 8
        and (total_j - sum(tail)) % t == 0
    ):
        sched = [t] * ((total_j - sum(tail)) // t) + tail
        n_main = len(sched) - len(tail)
    else:
        assert total_j % t == 0
        sched = [t] * (total_j // t)
        n_main = len(sched)
        tail_mode = 0
    assert sum(sched) == total_j
    n_super = len(sched)
    n_cnt = t // cc
    # chunk k is touched by supertiles with ts >= (k+1)*cc; remainder j's
    # (ts % cc != 0, only in deep-taper tails) get one-shot psum tiles
    cnt_last = {
        k: max(s for s, ts in enumerate(sched) if ts // cc > k)
        for k in range(n_cnt)
    }
    rem_tiles = [(s, sched[s] % cc) for s in range(n_super) if sched[s] % cc]
    cnt_extra_w = sum(r for _, r in rem_tiles) * C

    nc = bacc.Bacc(
        "TRN2", target_bir_lowering=False, debug=False, num_devices=N_CORES
    )

    feats = nc.dram_tensor(
        "features", [rows_per_core, D], F32, kind="ExternalInput"
    ).ap()
    labels = nc.dram_tensor(
        "labels", [rows_per_core, C], F32, kind="ExternalInput"
    ).ap()
    # two sums blocks when the tail uses its own accumulator (host adds them)
    n_sums = 2 if tail_mode == 2 else 1
    out_partial = nc.dram_tensor(
        "partial", [C, n_sums * D], F32, kind="ExternalOutput"
    ).ap()
    out_counts = nc.dram_tensor(
        "counts", [1, n_cnt * cc * C + cnt_extra_w], F32, kind="ExternalOutput"
    ).ap()
    out_sqacc = nc.dram_tensor(
        "sqacc", [P, n_super], F32, kind="ExternalOutput"
    ).ap()

    # Blocked row mapping per supertile: row = row0 + p*ts + j -> partition p
    # reads ts contiguous rows (one contiguous DRAM chunk per partition).

    with tile.TileContext(nc) as tc:
        with (
            tc.tile_pool(name="feat", bufs=bufs) as feat_pool,
            tc.tile_pool(name="lbl", bufs=bufs) as lbl_pool,
            tc.tile_pool(name="oh", bufs=3) as oh_pool,
            tc.tile_pool(name="sq", bufs=2) as sq_pool,
            tc.tile_pool(name="acc", bufs=1) as acc_pool,
            tc.tile_pool(name="ps", bufs=1, space="PSUM") as psum_pool,
        ):
            psum_sums = psum_pool.tile([C, D], F32, tag="ps_sums")
            if n_sums == 2:
                psum_sumsB = psum_pool.tile([C, D], F32, tag="ps_sumsB")
            psum_cnt = [
                psum_pool.tile(
                    [1, cc * C], F32, tag=f"ps_cnt{k}", name=f"ps_cnt{k}"
                )
                for k in range(n_cnt)
            ]
            psum_cnt_rem = {
                s: psum_pool.tile(
                    [1, r * C], F32, tag=f"ps_cntr{s}", name=f"ps_cntr{s}"
                )
                for s, r in rem_tiles
            }
            sqacc = acc_pool.tile([P, n_super], F32, tag="sqacc")
            ones = acc_pool.tile([P, 1], BF16, tag="ones")
            part_sb = acc_pool.tile([C, n_sums * D], F32, tag="part")
            cnt_sb = acc_pool.tile(
                [1, n_cnt * cc * C + cnt_extra_w], F32, tag="cnt"
            )
            nc.vector.memset(ones[:, :], 1.0)

            def make_onehot(lbl_ap, mx, oh, ts):
                nc.vector.reduce_max(
                    mx[:, :ts], lbl_ap, axis=mybir.AxisListType.X
                )
                mxb = mx[:, :ts].unsqueeze(-1).broadcast_to((P, ts, C))
                nc.vector.tensor_tensor(
                    out=oh[:, :ts, :], in0=lbl_ap, in1=mxb,
                    op=mybir.AluOpType.is_equal,
                )

            lbl_eng = nc.scalar if lbl_on_act else nc.sync

            # Tail one-hots are prepared mid-stream (see emit_tail_prep call
            # inside the loop): their labels and DVE work land while the
            # pipeline has slack, so after the final feature DMA only
            # matmuls + tiny copies remain. Emitting them at the very front
            # would delay supertile 0's matmuls and stall the feature
            # stream once the buffer pool fills.
            tail_oh = {}

            def emit_tail_prep():
                row0 = P * sum(sched[:n_main])
                for i in range(n_main, n_super):
                    ts = sched[i]
                    lv = labels[row0 : row0 + P * ts].rearrange(
                        "(p j) c -> p j c", p=P, j=ts
                    )
                    row0 += P * ts
                    lt = acc_pool.tile(
                        [P, ts, C], F32, tag=f"lblt{i}", name=f"lblt{i}"
                    )
                    mxt = acc_pool.tile(
                        [P, ts], F32, tag=f"mxt{i}", name=f"mxt{i}"
                    )
                    oht = acc_pool.tile(
                        [P, ts, C], BF16, tag=f"oht{i}", name=f"oht{i}"
                    )
                    lbl_eng.dma_start(out=lt[:, :, :], in_=lv)
                    make_onehot(lt[:, :, :], mxt, oht, ts)
                    tail_oh[i] = oht

            cnt_off = {}  # output column offset per (chunk or rem tile)
            off = 0
            for k in range(n_cnt):
                cnt_off[("k", k)] = off
                off += cc * C
            for s, r in rem_tiles:
                cnt_off[("r", s)] = off
                off += r * C

            row0 = 0
            for s, ts in enumerate(sched):
                fv = feats[row0 : row0 + P * ts].rearrange(
                    "(p j) d -> p j d", p=P, j=ts
                )
                lv = labels[row0 : row0 + P * ts].rearrange(
                    "(p j) c -> p j c", p=P, j=ts
                )
                row0 += P * ts

                feat_t = feat_pool.tile([P, t, D], BF16, tag="feat")
                if pool_cast:
                    feat32 = feat_pool.tile([P, t, D], F32, tag="feat32")
                    nc.sync.dma_start(out=feat32[:, :ts, :], in_=fv)
                    nc.gpsimd.tensor_copy(feat_t[:, :ts, :], feat32[:, :ts, :])
                    sq_in = feat32
                else:
                    # SWDGE (gpsimd) casts f32 -> bf16 during the transfer
                    nc.gpsimd.dma_start(out=feat_t[:, :ts, :], in_=fv)
                    sq_in = feat_t

                if s in tail_oh:
                    onehot = tail_oh[s]
                else:
                    lbl_t = lbl_pool.tile([P, t, C], F32, tag="lbl")
                    lbl_eng.dma_start(out=lbl_t[:, :ts, :], in_=lv)
                    mx = oh_pool.tile([P, t], F32, tag="mx")
                    onehot = oh_pool.tile([P, t, C], BF16, tag="oh")
                    make_onehot(lbl_t[:, :ts, :], mx, onehot, ts)

                sq_t = sq_pool.tile([P, t, D], BF16, tag="sq")
                nc.scalar.activation(
                    sq_t[:, :ts, :],
                    sq_in[:, :ts, :],
                    mybir.ActivationFunctionType.Square,
                    accum_out=sqacc[:, s : s + 1],
                )

                ps = psum_sums if (n_sums == 1 or s < n_main) else psum_sumsB
                first = s == 0 if (n_sums == 1 or s < n_main) else s == n_main
                last = (
                    s == (n_main - 1 if n_sums == 2 else n_super - 1)
                    if (n_sums == 1 or s < n_main)
                    else s == n_super - 1
                )
                for j in range(ts):
                    nc.tensor.matmul(
                        ps[:, :],
                        onehot[:, j],
                        feat_t[:, j],
                        start=(first and j == 0),
                        stop=(last and j == ts - 1),
                    )
                # counts: ones^T @ onehot -> column sums, per-(j,c)
                for k in range(ts // cc):
                    nc.tensor.matmul(
                        psum_cnt[k][:, :],
                        ones[:, :],
                        onehot[:, k * cc : (k + 1) * cc],
                        start=(s == 0),
                        stop=(s == cnt_last[k]),
                    )
                    if s == cnt_last[k]:
                        o = cnt_off[("k", k)]
                        nc.vector.tensor_copy(
                            cnt_sb[:, o : o + cc * C], psum_cnt[k][:, :]
                        )
                if s in psum_cnt_rem:
                    r = ts % cc
                    nc.tensor.matmul(
                        psum_cnt_rem[s][:, :],
                        ones[:, :],
                        onehot[:, ts - r : ts],
                        start=True,
                        stop=True,
                    )
                    o = cnt_off[("r", s)]
                    nc.vector.tensor_copy(
                        cnt_sb[:, o : o + r * C], psum_cnt_rem[s][:, :]
                    )
                # main sums block closes before the tail: read it out while
                # the tail supertiles stream
                if n_sums == 2 and s == n_main - 1:
                    nc.vector.tensor_copy(part_sb[:, :D], psum_sums[:, :])
                    nc.sync.dma_start(
                        out=out_partial[:, :D], in_=part_sb[:, :D]
                    )
                # prepare tail one-hots once the pipeline is rolling
                if tail_mode == 2 and s == min(2, n_main - 1):
                    emit_tail_prep()
                # all sqacc columns except the last ship early
                if s == n_super - 2 and n_super >= 2:
                    nc.sync.dma_start(
                        out=out_sqacc[:, : n_super - 1],
                        in_=sqacc[:, : n_super - 1],
                    )

            if n_sums == 2:
                nc.vector.tensor_copy(part_sb[:, D:], psum_sumsB[:, :])
                nc.sync.dma_start(out=out_partial[:, D:], in_=part_sb[:, D:])
            else:
                nc.vector.tensor_copy(part_sb[:, :], psum_sums[:, :])
                nc.sync.dma_start(out=out_partial[:, :], in_=part_sb[:, :])
            nc.sync.dma_start(out=out_counts[:, :], in_=cnt_sb[:, :])
            if n_super >= 2:
                nc.sync.dma_start(
                    out=out_sqacc[:, n_super - 1 :],
                    in_=sqacc[:, n_super - 1 :],
                )
            else:
                nc.sync.dma_start(out=out_sqacc[:, :], in_=sqacc[:, :])

    nc.compile()
    return nc


_NC_CACHE: dict = {}


def _get_nc():
    if "nc" not in _NC_CACHE:
        _NC_CACHE["nc"] = build_nc(N_TOTAL // N_CORES)
    return _NC_CACHE["nc"]


def finalize(partials, countss, sqaccs):
    """Host gather/unshard: combine per-core partials into the scalar loss."""
    sums = np.zeros((C, D), np.float64)
    counts = np.zeros((C,), np.float64)
    sumsq = 0.0
    for part, cnt, sq in zip(partials, countss, sqaccs):
        sums += part.astype(np.float64).reshape(C, -1, D).sum(axis=1)
        counts += cnt.astype(np.float64).reshape(-1, C).sum(axis=0)
        sumsq += float(sq.astype(np.float64).sum())
    centers = (
        np.where(counts[:, None] > 0, sums / np.maximum(counts, 1.0)[:, None], 0.0)
        + 1e-6
    )
    intra = (
        sumsq
        - 2.0 * float((sums * centers).sum())
        + float((counts * (centers**2).sum(axis=1)).sum())
    )
    cmean = centers.mean(axis=0, keepdims=True)
    inter = float(((centers - cmean) ** 2).sum()) / C
    loss = intra / (inter + 1e-6)
    return np.array(loss, dtype=np.float32)


def kernel(features: np.ndarray, labels: np.ndarray) -> np.ndarray:
    features = np.asarray(features)
    labels = np.asarray(labels)
    assert features.shape == (N_TOTAL, D), features.shape
    assert labels.shape == (N_TOTAL, C), labels.shape
    nc = _get_nc()
    rows = N_TOTAL // N_CORES
    in_maps = []
    for i in range(N_CORES):
        sl = slice(i * rows, (i + 1) * rows)
        in_maps.append(
            {
                "features": np.ascontiguousarray(features[sl], dtype=np.float32),
                "labels": np.ascontiguousarray(labels[sl], dtype=np.float32),
            }
        )
    res = run_bass_kernel_spmd(nc, in_maps, list(range(N_CORES)))
    return finalize(
        [r["partial"] for r in res.results],
        [r["counts"] for r in res.results],
        [r["sqacc"] for r in res.results],
    )


# revision 36
# speedup vs baseline: 1.1610x; 1.1610x over previous
"""AffinityLoss (segment-reduce) Trainium2 kernel.

Math (single pass over the data — no per-row center gather needed):
    lbl     = argmax(labels, axis=1)                         (N,)
    sums_c  = sum of features rows with lbl == c             (C, D)
    n_c     = count of rows with lbl == c                    (C,)
    sumsq   = sum(features ** 2)                             scalar
    centers = where(n>0, sums/max(n,1), 0) + 1e-6
    intra   = sumsq - 2*sum(sums*centers) + sum(n_c*||c_c||^2)
    inter   = sum((centers - mean(centers))^2) / C
    loss    = intra / (inter + 1e-6)

Per core (data-parallel over N):
  - one-hot(argmax) built on the vector engine (reduce_max + one
    broadcast is_equal over the whole supertile)
  - segment sums via PE: one matmul per 128-row group
    (one-hot^T @ features) accumulated in PSUM over the full loop
  - counts via PE with ones as the stationary operand
    (ones^T @ one-hot -> per-(j,c) column counts)
  - sum-of-squares on the scalar engine (Square activation + accumulate)
Features stream as f32 -> bf16 cast DMAs (SWDGE), contiguous per
partition per supertile; the supertile schedule tapers at the end so the
compute tail after the last DMA is short. The O(C*D) finalization runs
on the host over the 8 per-core partials (the gather/unshard step).

Measured: ~131us HW exec on 8 cores (pure-DMA floor for the same
45.25MB/core stream: ~125us; HBM roofline 45.25MB / 358GB/s = 126us).
"""

import numpy as np

import concourse.bacc as bacc
import concourse.tile as tile
from concourse import mybir
from concourse.bass_utils import run_bass_kernel_spmd

N_CORES = 8
N_TOTAL = 262144
D = 256
C = 100
P = 128
T = 16  # 128-row groups per supertile (DMA batch)

F32 = mybir.dt.float32
BF16 = mybir.dt.bfloat16


def build_nc(
    rows_per_core: int,
    t: int = T,
    bufs: int = 6,
    tail_mode: int = 3,
    pool_cast: bool = False,
    early_copies: bool = False,
    lbl_on_act: bool = False,
):
    """Build the per-core Bass program (same SPMD program on all cores).

    tail_mode: 0 = uniform supertiles; 1 = tapered tail [t/2,t/4,t/4];
    3 = deep taper [t/2,t/4,t/8,t/8] (default; tiny tiles use one-shot
    counts matmuls); 2 = mode 3 plus up-front tail one-hots and split sums
    PSUM — measured consistently slower, kept for reference.
    """
    total_j = rows_per_core // P
    cc = 4  # j's per counts matmul (free dim cc*C <= 512)
    assert t % cc == 0
    # Supertile schedule: mostly t, tapering at the end so the compute tail
    # after the final DMA is short.
    if tail_mode >= 2 and t % 8 == 0:
        tail = [t // 2, t // 4, t // 8, t // 8]
    elif t // 4 >= cc:
        tail = [t // 2, t // 4, t // 4]
    else:
        tail = [t // 2, t // 2]
    if (
        tail_mode > 0
        and total_j > 2 * t
        and t >= 8
        and (total_j - sum(tail)) % t == 0
    ):
        sched = [t] * ((total_j - sum(tail)) // t) + tail
        n_main = len(sched) - len(tail)
    else:
        assert total_j % t == 0
        sched = [t] * (total_j // t)
        n_main = len(sched)
        tail_mode = 0
    assert sum(sched) == total_j
    n_super = len(sched)
    n_cnt = t // cc
    # chunk k is touched by supertiles with ts >= (k+1)*cc; remainder j's
    # (ts % cc != 0, only in deep-taper tails) get one-shot psum tiles
    cnt_last = {
        k: max(s for s, ts in enumerate(sched) if ts // cc > k)
        for k in range(n_cnt)
    }
    rem_tiles = [(s, sched[s] % cc) for s in range(n_super) if sched[s] % cc]
    cnt_extra_w = sum(r for _, r in rem_tiles) * C

    nc = bacc.Bacc(
        "TRN2", target_bir_lowering=False, debug=False, num_devices=N_CORES
    )

    feats = nc.dram_tensor(
        "features", [rows_per_core, D], F32, kind="ExternalInput"
    ).ap()
    labels = nc.dram_tensor(
        "labels", [rows_per_core, C], F32, kind="ExternalInput"
    ).ap()
    # two sums blocks when the tail uses its own accumulator (host adds them)
    n_sums = 2 if tail_mode == 2 else 1
    out_partial = nc.dram_tensor(
        "partial", [C, n_sums * D], F32, kind="ExternalOutput"
    ).ap()
    out_counts = nc.dram_tensor(
        "counts", [1, n_cnt * cc * C + cnt_extra_w], F32, kind="ExternalOutput"
    ).ap()
    out_sqacc = nc.dram_tensor(
        "sqacc", [P, n_super], F32, kind="ExternalOutput"
    ).ap()

    # Blocked row mapping per supertile: row = row0 + p*ts + j -> partition p
    # reads ts contiguous rows (one contiguous DRAM chunk per partition).

    with tile.TileContext(nc) as tc:
        with (
            tc.tile_pool(name="feat", bufs=bufs) as feat_pool,
            tc.tile_pool(name="lbl", bufs=bufs) as lbl_pool,
            tc.tile_pool(name="oh", bufs=3) as oh_pool,
            tc.tile_pool(name="sq", bufs=2) as sq_pool,
            tc.tile_pool(name="acc", bufs=1) as acc_pool,
            tc.tile_pool(name="ps", bufs=1, space="PSUM") as psum_pool,
        ):
            psum_sums = psum_pool.tile([C, D], F32, tag="ps_sums")
            if n_sums == 2:
                psum_sumsB = psum_pool.tile([C, D], F32, tag="ps_sumsB")
            psum_cnt = [
                psum_pool.tile(
                    [1, cc * C], F32, tag=f"ps_cnt{k}", name=f"ps_cnt{k}"
                )
                for k in range(n_cnt)
            ]
            psum_cnt_rem = {
                s: psum_pool.tile(
                    [1, r * C], F32, tag=f"ps_cntr{s}", name=f"ps_cntr{s}"
                )
                for s, r in rem_tiles
            }
            sqacc = acc_pool.tile([P, n_super], F32, tag="sqacc")
            ones = acc_pool.tile([P, 1], BF16, tag="ones")
            part_sb = acc_pool.tile([C, n_sums * D], F32, tag="part")
            cnt_sb = acc_pool.tile(
                [1, n_cnt * cc * C + cnt_extra_w], F32, tag="cnt"
            )
            nc.vector.memset(ones[:, :], 1.0)

            def make_onehot(lbl_ap, mx, oh, ts):
                nc.vector.reduce_max(
                    mx[:, :ts], lbl_ap, axis=mybir.AxisListType.X
                )
                mxb = mx[:, :ts].unsqueeze(-1).broadcast_to((P, ts, C))
                nc.vector.tensor_tensor(
                    out=oh[:, :ts, :], in0=lbl_ap, in1=mxb,
                    op=mybir.AluOpType.is_equal,
                )

            lbl_eng = nc.scalar if lbl_on_act else nc.sync

            # Tail one-hots are prepared mid-stream (see emit_tail_prep call
            # inside the loop): their labels and DVE work land while the
            # pipeline has slack, so after the final feature DMA only
            # matmuls + tiny copies remain. Emitting them at the very front
            # would delay supertile 0's matmuls and stall the feature
            # stream once the buffer pool fills.
            tail_oh = {}

            def emit_tail_prep():
                row0 = P * sum(sched[:n_main])
                for i in range(n_main, n_super):
                    ts = sched[i]
                    lv = labels[row0 : row0 + P * ts].rearrange(
                        "(p j) c -> p j c", p=P, j=ts
                    )
                    row0 += P * ts
                    lt = acc_pool.tile(
                        [P, ts, C], F32, tag=f"lblt{i}", name=f"lblt{i}"
                    )
                    mxt = acc_pool.tile(
                        [P, ts], F32, tag=f"mxt{i}", name=f"mxt{i}"
                    )
                    oht = acc_pool.tile(
                        [P, ts, C], BF16, tag=f"oht{i}", name=f"oht{i}"
                    )
                    lbl_eng.dma_start(out=lt[:, :, :], in_=lv)
                    make_onehot(lt[:, :, :], mxt, oht, ts)
                    tail_oh[i] = oht

            cnt_off = {}  # output column offset per (chunk or rem tile)
            off = 0
            for k in range(n_cnt):
                cnt_off[("k", k)] = off
                off += cc * C
            for s, r in rem_tiles:
                cnt_off[("r", s)] = off
                off += r * C

            row0 = 0
            for s, ts in enumerate(sched):
                fv = feats[row0 : row0 + P * ts].rearrange(
                    "(p j) d -> p j d", p=P, j=ts
                )
                lv = labels[row0 : row0 + P * ts].rearrange(
                    "(p j) c -> p j c", p=P, j=ts
                )
                row0 += P * ts

                feat_t = feat_pool.tile([P, t, D], BF16, tag="feat")
                if pool_cast:
                    feat32 = feat_pool.tile([P, t, D], F32, tag="feat32")
                    nc.sync.dma_start(out=feat32[:, :ts, :], in_=fv)
                    nc.gpsimd.tensor_copy(feat_t[:, :ts, :], feat32[:, :ts, :])
                    sq_in = feat32
                else:
                    # SWDGE (gpsimd) casts f32 -> bf16 during the transfer
                    nc.gpsimd.dma_start(out=feat_t[:, :ts, :], in_=fv)
                    sq_in = feat_t

                if s in tail_oh:
                    onehot = tail_oh[s]
                else:
                    lbl_t = lbl_pool.tile([P, t, C], F32, tag="lbl")
                    lbl_eng.dma_start(out=lbl_t[:, :ts, :], in_=lv)
                    mx = oh_pool.tile([P, t], F32, tag="mx")
                    onehot = oh_pool.tile([P, t, C], BF16, tag="oh")
                    make_onehot(lbl_t[:, :ts, :], mx, onehot, ts)

                sq_t = sq_pool.tile([P, t, D], BF16, tag="sq")
                nc.scalar.activation(
                    sq_t[:, :ts, :],
                    sq_in[:, :ts, :],
                    mybir.ActivationFunctionType.Square,
                    accum_out=sqacc[:, s : s + 1],
                )

                ps = psum_sums if (n_sums == 1 or s < n_main) else psum_sumsB
                first = s == 0 if (n_sums == 1 or s < n_main) else s == n_main
                last = (
                    s == (n_main - 1 if n_sums == 2 else n_super - 1)
                    if (n_sums == 1 or s < n_main)
                    else s == n_super - 1
                )
                for j in range(ts):
                    nc.tensor.matmul(
                        ps[:, :],
                        onehot[:, j],
                        feat_t[:, j],
                        start=(first and j == 0),
                        stop=(last and j == ts - 1),
                    )
                # counts: ones^T @ onehot -> column sums, per-(j,c)
                for k in range(ts // cc):
                    nc.tensor.matmul(
                        psum_cnt[k][:, :],
                        ones[:, :],
                        onehot[:, k * cc : (k + 1) * cc],
                        start=(s == 0),
                        stop=(s == cnt_last[k]),
                    )
                    if early_copies and s == cnt_last[k]:
                        o = cnt_off[("k", k)]
                        nc.vector.tensor_copy(
                            cnt_sb[:, o : o + cc * C], psum_cnt[k][:, :]
                        )
                if s in psum_cnt_rem:
                    r = ts % cc
                    nc.tensor.matmul(
                        psum_cnt_rem[s][:, :],
                        ones[:, :],
                        onehot[:, ts - r : ts],
                        start=True,
                        stop=True,
                    )
                    if early_copies:
                        o = cnt_off[("r", s)]
                        nc.vector.tensor_copy(
                            cnt_sb[:, o : o + r * C], psum_cnt_rem[s][:, :]
                        )
                # main sums block closes before the tail: read it out while
                # the tail supertiles stream
                if n_sums == 2 and s == n_main - 1:
                    nc.vector.tensor_copy(part_sb[:, :D], psum_sums[:, :])
                    nc.sync.dma_start(
                        out=out_partial[:, :D], in_=part_sb[:, :D]
                    )
                # prepare tail one-hots once the pipeline is rolling
                if tail_mode == 2 and s == min(2, n_main - 1):
                    emit_tail_prep()
                # all sqacc columns except the last ship early
                if s == n_super - 2 and n_super >= 2:
                    nc.sync.dma_start(
                        out=out_sqacc[:, : n_super - 1],
                        in_=sqacc[:, : n_super - 1],
                    )

            if n_sums == 2:
                nc.vector.tensor_copy(part_sb[:, D:], psum_sumsB[:, :])
                nc.sync.dma_start(out=out_partial[:, D:], in_=part_sb[:, D:])
            else:
                nc.vector.tensor_copy(part_sb[:, :], psum_sums[:, :])
                nc.sync.dma_start(out=out_partial[:, :], in_=part_sb[:, :])
            if not early_copies:
                for k in range(n_cnt):
                    o = cnt_off[("k", k)]
                    nc.vector.tensor_copy(
                        cnt_sb[:, o : o + cc * C], psum_cnt[k][:, :]
                    )
                for sr, r in rem_tiles:
                    o = cnt_off[("r", sr)]
                    nc.vector.tensor_copy(
                        cnt_sb[:, o : o + r * C], psum_cnt_rem[sr][:, :]
                    )
            nc.sync.dma_start(out=out_counts[:, :], in_=cnt_sb[:, :])
            if n_super >= 2:
                nc.sync.dma_start(
                    out=out_sqacc[:, n_super - 1 :],
                    in_=sqacc[:, n_super - 1 :],
                )
            else:
                nc.sync.dma_start(out=out_sqacc[:, :], in_=sqacc[:, :])

    nc.compile()
    return nc


_NC_CACHE: dict = {}


def _get_nc():
    if "nc" not in _NC_CACHE:
        _NC_CACHE["nc"] = build_nc(N_TOTAL // N_CORES)
    return _NC_CACHE["nc"]


def finalize(partials, countss, sqaccs):
    """Host gather/unshard: combine per-core partials into the scalar loss."""
    sums = np.zeros((C, D), np.float64)
    counts = np.zeros((C,), np.float64)
    sumsq = 0.0
    for part, cnt, sq in zip(partials, countss, sqaccs):
        sums += part.astype(np.float64).reshape(C, -1, D).sum(axis=1)
        counts += cnt.astype(np.float64).reshape(-1, C).sum(axis=0)
        sumsq += float(sq.astype(np.float64).sum())
    centers = (
        np.where(counts[:, None] > 0, sums / np.maximum(counts, 1.0)[:, None], 0.0)
        + 1e-6
    )
    intra = (
        sumsq
        - 2.0 * float((sums * centers).sum())
        + float((counts * (centers**2).sum(axis=1)).sum())
    )
    cmean = centers.mean(axis=0, keepdims=True)
    inter = float(((centers - cmean) ** 2).sum()) / C
    loss = intra / (inter + 1e-6)
    return np.array(loss, dtype=np.float32)


def kernel(features: np.ndarray, labels: np.ndarray) -> np.ndarray:
    features = np.asarray(features)
    labels = np.asarray(labels)
    assert features.shape == (N_TOTAL, D), features.shape
    assert labels.shape == (N_TOTAL, C), labels.shape
    nc = _get_nc()
    rows = N_TOTAL // N_CORES
    in_maps = []
    for i in range(N_CORES):
        sl = slice(i * rows, (i + 1) * rows)
        in_maps.append(
            {
                "features": np.ascontiguousarray(features[sl], dtype=np.float32),
                "labels": np.ascontiguousarray(labels[sl], dtype=np.float32),
            }
        )
    res = run_bass_kernel_spmd(nc, in_maps, list(range(N_CORES)))
    return finalize(
        [r["partial"] for r in res.results],
        [r["counts"] for r in res.results],
        [r["sqacc"] for r in res.results],
    )
